# revision 33
# baseline (speedup 1.0000x reference)
"""CausalLocalSGU Trainium2 kernel.

Reference computation (per batch b):
  split x[b] channels -> res (first 1024), gate_in (last 1024)
  per 128-token window block j: z_j = LayerNorm(gate_in_j) * gamma + beta
  gate_out_j[m, c] = sum_n W[h(c), m, n] * [z_{j-1}; z_j][n, c] + bias[h(c), m]
      (W masked causally: keep [m, n] where n <= m + 128; z_{-1} = 0)
  out_j = gate_out_j * res_j

Sharding: 8 cores; core k handles batch k//2, token half k%2 (2048 tokens =
16 window blocks) plus a one-block halo on the left (zeros for even cores).
Halo LNs are recomputed locally -> no collectives.

Precision: gate half is cast to fp8-e4m3 on the host (it only feeds the
~7e-5-magnitude SGU einsum term; weights ~1e-5).  res and out travel as
bf16 (~0.2% rel err; tolerance is 2e-2); the host upcasts the output to
fp32.  LN stats are estimated from every other channel (512 samples; the
~2% rstd estimate error perturbs the output by ~1e-6 relative, far below
the fp8-gate noise).  This cuts per-core HBM traffic from 19.3 MB to
10.6 MB and halves the DVE bn_stats cost.

Structure: each core runs its 16 blocks as TWO independent 8-block chains
(the second chain re-LNs gate block 8 as its halo).  The per-block
dependency chain (stats->aggr->rstd->negmu->znorm->matmuls->drain->mult->
store) is ~6 us deep; two interleaved chains keep every engine fed and
halve pipeline fill/drain.  Per block [128,1024]:
  DVE:  bn_stats (subsampled, 1x-only op) + bn_aggr + negmu + one
        macro-wide combine multiply per 4 blocks (t_t bf16 2x mode),
        issued one step late so it never waits in the in-order stream
  ACT:  rstd (Abs_reciprocal_sqrt) + psum drains (+bias imm, fp32->bf16)
  Gp:   all LN-normalizes (fp8-in tensor_scalar mult/add w/ dense [W,1]
        scalar tiles -- the only fast Q7 form; Pool has no PSUM access)
  PE:   8 matmuls + 8 ldweights per block, bf16 (z bf16 x wt bf16)
DMA: all loads and stores ride the sync HWDGE ring as flat 2D copies of
host pre-interleaved partition-major [W, blocks*DOUT] arrays (constant
descriptor issue cost); store issue on gpsimd would convoy the znorms.

Fast path requires gamma == ones, beta == zeros and a uniform bias;
anything else compiles the general variant (full-width stats, fp32
extras matmul carrying bias + S*beta, explicit gamma multiply, fp32
res/out, single chain).
"""

import ml_dtypes
import numpy as np

import concourse.bacc as bacc
import concourse.bass as bass
import concourse.tile as tile
from concourse import mybir
from concourse.bass_utils import run_bass_kernel_spmd

F32 = mybir.dt.float32
BF16 = mybir.dt.bfloat16
FP8 = mybir.dt.float8e4

HEADS = 4
W = 128            # window
DIM = 2048
DOUT = 1024        # dim // 2
DHEAD = DOUT // HEADS  # 256
B = 4
N = 4096
NCORES = 8
BLK_PER_CORE = (N // 2) // W   # 16
MACRO = 4          # window blocks per output store batch
CHAIN = 8          # blocks per independent pipeline chain
LN_EPS = 1e-5

# fp32 consts layout ([4, 1536]): K=4 extras matmul operands (general path).
_EXR0 = 0           # [4, 256]: lhsT, halves 0,1 (S = S_full)
_EXF0 = 256         # [4, 256]: lhsT, halves 0,1 (S = S_first)
_RHSX0 = 512        # [4, 1024]: rhs for half 0 then half 1
_CONSTS_COLS = 1536

_NC_CACHE: dict = {}
_last_in_maps: list = []


def _build_nc(general: bool, bias_val: float = 1.0) -> bass.Bass:
    nc = bacc.Bacc(
        trn_type="TRN2",
        target_bir_lowering=False,
        debug=False,
        num_devices=NCORES,
    )
    nblk = BLK_PER_CORE  # output blocks per core; +1 halo block for gate
    res_dt = F32 if general else BF16
    # partition-major layouts ([W, blocks*DOUT], host pre-interleaved):
    # every DMA is a flat contiguous 2D copy
    res_sh = nc.dram_tensor("res_sh", [W, nblk * DOUT], res_dt,
                            kind="ExternalInput").ap()
    gate_sh = nc.dram_tensor(
        "gate_sh", [W, (nblk + 1) * DOUT], FP8, kind="ExternalInput"
    ).ap()
    consts4 = nc.dram_tensor(
        "consts4", [4, _CONSTS_COLS], F32, kind="ExternalInput"
    ).ap()
    consts_bf = nc.dram_tensor(
        "consts_bf", [W, 2 * HEADS * W], BF16, kind="ExternalInput"
    ).ap()
    if general:
        gamma = nc.dram_tensor("gamma", [DOUT], F32, kind="ExternalInput").ap()
    out = nc.dram_tensor("out", [W, nblk * DOUT], res_dt,
                         kind="ExternalOutput").ap()

    ident = mybir.ActivationFunctionType.Identity
    alu = mybir.AluOpType

    with tile.TileContext(nc) as tc:
        with (
            tc.tile_pool(name="singles", bufs=1) as singles,
            tc.tile_pool(name="gpool", bufs=1) as gpool,
            tc.tile_pool(name="rpool", bufs=4) as rpool,
            tc.tile_pool(name="opool", bufs=4) as opool,
            tc.tile_pool(name="zpool", bufs=8) as zpool,
            tc.tile_pool(name="cpool", bufs=4) as cpool,
            tc.tile_pool(name="spool", bufs=16) as spool,
            tc.tile_pool(name="ppool", bufs=4, space="PSUM") as ppool,
        ):
            consts4_t = singles.tile([4, _CONSTS_COLS], F32)
            wt_t = singles.tile([W, 2 * HEADS * W], BF16)
            eps_t = singles.tile([128, 1], F32)
            nc.vector.memset(eps_t, LN_EPS)
            if general:
                gamma_t = singles.tile([128, DOUT], F32)

            # halo block load first (smallest, unblocks chain A)
            gate0 = gpool.tile([W, DOUT], FP8, tag="gate0")
            nc.sync.dma_start(out=gate0, in_=gate_sh[:, 0:DOUT])
            if general:
                nc.gpsimd.dma_start(
                    out=gamma_t,
                    in_=bass.AP(
                        tensor=gamma.tensor,
                        offset=gamma.offset,
                        ap=[[0, 128]] + list(gamma.ap),
                    ),
                )
            exr_t = consts4_t[:, _EXR0 : _EXR0 + 2 * W]
            exf_t = consts4_t[:, _EXF0 : _EXF0 + 2 * W]
            rhsx_t = consts4_t[:, _RHSX0 : _RHSX0 + DOUT]

            # sync-ring order: small early gate chunks unblock both
            # chains' LN heads, then bulk; res macros land just in time
            nmac = nblk // MACRO
            gseg = []
            r4s = [None] * nmac

            def load_g(lo, n):
                gt = gpool.tile([W, n * DOUT], FP8, tag=f"g{lo}")
                nc.sync.dma_start(
                    out=gt, in_=gate_sh[:, lo * DOUT : (lo + n) * DOUT]
                )
                gseg.append((lo, n, gt))

            def load_r4(m):
                r4 = rpool.tile([W, MACRO * DOUT], res_dt, tag="r4")
                nc.sync.dma_start(
                    out=r4,
                    in_=res_sh[:, m * MACRO * DOUT : (m + 1) * MACRO * DOUT],
                )
                r4s[m] = r4

            load_g(1, 2)       # chain A head
            load_g(8, 2)       # chain B head (halo 8 + block 9)
            nc.sync.dma_start(out=wt_t, in_=consts_bf)
            load_g(3, 5)       # chain A bulk
            load_g(10, 7)      # chain B bulk (both chains eat gate early)
            nc.sync.dma_start(out=consts4_t, in_=consts4)
            load_r4(0)
            load_r4(2)
            load_r4(1)
            load_r4(3)

            def gate_ap(gb):
                if gb == 0:
                    return gate0
                for lo, n, gt in gseg:
                    if lo <= gb < lo + n:
                        return gt[:, (gb - lo) * DOUT : (gb - lo + 1) * DOUT]
                raise AssertionError(gb)

            def res_ap(blk, n=1):
                lo = (blk % MACRO) * DOUT
                return r4s[blk // MACRO][:, lo : lo + n * DOUT]

            def ln_stats(gate):
                """stage 1: bn stats + rstd request (DVE + ACT)."""
                if general:
                    stats = spool.tile([W, 2, 6], F32, tag="stats")
                    nc.vector.bn_stats(out=stats[:, 0], in_=gate[:, :512])
                    nc.vector.bn_stats(out=stats[:, 1], in_=gate[:, 512:])
                else:
                    stats = spool.tile([W, 6], F32, tag="stats")
                    nc.vector.bn_stats(out=stats, in_=gate[:, 0:DOUT:2])
                mv = spool.tile([W, 2], F32, tag="mv")
                nc.vector.bn_aggr(out=mv, in_=stats)
                rstd = spool.tile([W, 1], F32, tag="rstd")
                nc.scalar.activation(
                    out=rstd,
                    in_=mv[:, 1:2],
                    func=mybir.ActivationFunctionType.Abs_reciprocal_sqrt,
                    bias=eps_t,
                )
                return mv, rstd

            def ln_norm(gate, mv, rstd):
                """stage 2: normalize into a bf16 z tile (GpSimd)."""
                negmu = spool.tile([W, 1], F32, tag="negmu")
                nc.vector.tensor_scalar(
                    out=negmu,
                    in0=mv[:, 0:1],
                    scalar1=rstd,
                    scalar2=-1.0,
                    op0=alu.mult,
                    op1=alu.mult,
                )
                z = zpool.tile([W, DOUT], BF16, tag="z")
                if not general:
                    nc.gpsimd.tensor_scalar(
                        out=z, in0=gate, scalar1=rstd, scalar2=negmu,
                        op0=alu.mult, op1=alu.add,
                    )
                    return z
                nc.scalar.activation(
                    out=z, in_=gate, func=ident, bias=negmu, scale=rstd
                )
                nc.vector.tensor_mul(z, z, gamma_t)
                return z

            def store_macro(mq, o4, last):
                if not last:
                    nc.sync.dma_start(
                        out=out[:, mq * MACRO * DOUT : (mq + 1) * MACRO * DOUT],
                        in_=o4,
                    )
                    return
                # chain-final macro ships pair + singles: short store tail
                for lo, n in [(0, 2), (2, 1), (3, 1)]:
                    nc.sync.dma_start(
                        out=out[:, (mq * MACRO + lo) * DOUT
                                : (mq * MACRO + lo + n) * DOUT],
                        in_=o4[:, lo * DOUT : (lo + n) * DOUT],
                    )

            if general:
                # single-chain reference-shaped loop (correctness fallback)
                lnq = [ln_stats(gate_ap(0)), ln_stats(gate_ap(1))]
                z_prev = None
                o4 = None
                for gb in range(nblk + 1):
                    if gb + 2 <= nblk:
                        lnq.append(ln_stats(gate_ap(gb + 2)))
                    blk = gb - 1
                    if blk >= 0 and blk % MACRO == 0:
                        o4 = opool.tile([W, MACRO * DOUT], res_dt, tag="o4")
                    mv_c, rstd_c = lnq.pop(0)
                    z = ln_norm(gate_ap(gb), mv_c, rstd_c)
                    if blk >= 0:
                        s = blk % MACRO
                        psum = ppool.tile([W, DOUT], F32, tag="psum")
                        ex_t = exf_t if blk == 0 else exr_t
                        for u in range(2):
                            nc.tensor.matmul(
                                psum[:, u * 512 : (u + 1) * 512],
                                ex_t[:, u * W : (u + 1) * W],
                                rhsx_t[:, u * 512 : (u + 1) * 512],
                                start=True,
                                stop=False,
                            )
                            for h in (2 * u, 2 * u + 1):
                                ps = psum[:, h * DHEAD : (h + 1) * DHEAD]
                                nc.tensor.matmul(
                                    ps,
                                    wt_t[:, (2 * h) * W : (2 * h + 1) * W],
                                    z_prev[:, h * DHEAD : (h + 1) * DHEAD],
                                    start=False,
                                    stop=False,
                                )
                                nc.tensor.matmul(
                                    ps,
                                    wt_t[:, (2 * h + 1) * W : (2 * h + 2) * W],
                                    z[:, h * DHEAD : (h + 1) * DHEAD],
                                    start=False,
                                    stop=(h == 2 * u + 1),
                                )
                        nc.vector.tensor_mul(
                            o4[:, s * DOUT : (s + 1) * DOUT], psum, res_ap(blk)
                        )
                        if s == MACRO - 1:
                            store_macro(blk // MACRO, o4, last=False)
                    z_prev = z
            else:
                # two interleaved independent chains (c = 0, 1); chain c
                # covers output blocks 8c..8c+7, LN steps j = 0..8 map to
                # gate blocks 8c + j (j = 0 is the chain halo)
                LA = 3  # per-chain stats lookahead (6 in absolute ops)
                st = [
                    {"lnq": [], "z_prev": None, "o4": None, "c16": None}
                    for _ in range(2)
                ]
                for j in range(LA):
                    for c in range(2):
                        st[c]["lnq"].append(ln_stats(gate_ap(CHAIN * c + j)))
                pend = []      # delayed macro mult+store work

                def do_macro(mq, m_c16, m_o4, last):
                    if not last:
                        nc.vector.tensor_mul(m_o4, m_c16, r4s[mq])
                        store_macro(mq, m_o4, last)
                        return
                    # chain-final macro: mult+store in pair/single pieces
                    # so the tail is one short chain, not a 4-block one
                    for lo, n in [(0, 2), (2, 1), (3, 1)]:
                        sl = slice(lo * DOUT, (lo + n) * DOUT)
                        nc.vector.tensor_mul(
                            m_o4[:, sl], m_c16[:, sl], r4s[mq][:, sl]
                        )
                        nc.sync.dma_start(
                            out=out[:, (mq * MACRO + lo) * DOUT
                                    : (mq * MACRO + lo + n) * DOUT],
                            in_=m_o4[:, sl],
                        )
                for j in range(CHAIN + 1):
                    for c in range(2):
                        S = st[c]
                        # flush a completed macro's multiply+store one slot
                        # late: its c16 inputs are surely drained, so the
                        # wide DVE t_t never stalls the in-order stream
                        while pend:
                            mq, m_c16, m_o4, last = pend.pop(0)
                            do_macro(mq, m_c16, m_o4, last)
                        if j + LA <= CHAIN:
                            S["lnq"].append(
                                ln_stats(gate_ap(CHAIN * c + j + LA))
                            )
                        blk = CHAIN * c + j - 1
                        mv_c, rstd_c = S["lnq"].pop(0)
                        psum = None
                        if j >= 1:
                            if (j - 1) % MACRO == 0:
                                S["o4"] = opool.tile(
                                    [W, MACRO * DOUT], res_dt, tag="o4",
                                    name=f"o4_{c}_{j}",
                                )
                                S["c16"] = cpool.tile(
                                    [W, MACRO * DOUT], BF16, tag="c16",
                                    name=f"c16_{c}_{j}",
                                )
                            # prev-window matmuls only need z_prev: the PE
                            # works while this block's znorm is in flight
                            psum = ppool.tile([W, DOUT], F32, tag="psum")
                            for h in range(HEADS):
                                nc.tensor.matmul(
                                    psum[:, h * DHEAD : (h + 1) * DHEAD],
                                    wt_t[:, (2 * h) * W : (2 * h + 1) * W],
                                    S["z_prev"][:, h * DHEAD : (h + 1) * DHEAD],
                                    start=True,
                                    stop=False,
                                )
                        z = ln_norm(gate_ap(CHAIN * c + j), mv_c, rstd_c)
                        if j >= 1:
                            for h in range(HEADS):
                                nc.tensor.matmul(
                                    psum[:, h * DHEAD : (h + 1) * DHEAD],
                                    wt_t[:, (2 * h + 1) * W : (2 * h + 2) * W],
                                    z[:, h * DHEAD : (h + 1) * DHEAD],
                                    start=False,
                                    stop=True,
                                )
                            s = (j - 1) % MACRO
                            nc.scalar.activation(
                                out=S["c16"][:, s * DOUT : (s + 1) * DOUT],
                                in_=psum, func=ident,
                                bias=float(bias_val), scale=1.0,
                            )
                            if s == MACRO - 1:
                                pend.append((
                                    blk // MACRO, S["c16"], S["o4"],
                                    j == CHAIN,
                                ))
                        S["z_prev"] = z
                while pend:
                    mq, m_c16, m_o4, last = pend.pop(0)
                    do_macro(mq, m_c16, m_o4, last)
    if not nc.is_finalized():
        nc.finalize()
    return nc


def _host_prep(weight, bias, ln_beta):
    j = np.arange(2 * W)[None, :]
    i_ = np.arange(W)[:, None]
    mask = (j <= i_ + W).astype(np.float32)          # [W, 2W]
    wm = weight * mask[None]                         # [H, W, 2W]
    wT = np.zeros((W, 2 * HEADS, W), dtype=np.float32)
    for h in range(HEADS):
        wT[:, 2 * h] = wm[h, :, :W].T                # A_h: prev-window cols
        wT[:, 2 * h + 1] = wm[h, :, W:].T            # B_h: current-window cols
    wT = wT.reshape(W, 2 * HEADS * W)

    s_full = wm.sum(-1)                              # [H, W]
    s_first = wm[:, :, W:].sum(-1)

    def consts_for(first_has_prev: bool):
        c = np.zeros((4, _CONSTS_COLS), dtype=np.float32)
        sf = s_full if first_has_prev else s_first
        for u in range(2):
            # lhsT rows: bias[2u], S[2u], bias[2u+1], S[2u+1]
            c[0, _EXR0 + u * W : _EXR0 + (u + 1) * W] = bias[2 * u]
            c[1, _EXR0 + u * W : _EXR0 + (u + 1) * W] = s_full[2 * u]
            c[2, _EXR0 + u * W : _EXR0 + (u + 1) * W] = bias[2 * u + 1]
            c[3, _EXR0 + u * W : _EXR0 + (u + 1) * W] = s_full[2 * u + 1]
            c[0, _EXF0 + u * W : _EXF0 + (u + 1) * W] = bias[2 * u]
            c[1, _EXF0 + u * W : _EXF0 + (u + 1) * W] = sf[2 * u]
            c[2, _EXF0 + u * W : _EXF0 + (u + 1) * W] = bias[2 * u + 1]
            c[3, _EXF0 + u * W : _EXF0 + (u + 1) * W] = sf[2 * u + 1]
            # rhs rows: ind[2u], beta*ind[2u], ind[2u+1], beta*ind[2u+1]
            base = _RHSX0 + u * 512
            beta_u = ln_beta[u * 512 : (u + 1) * 512]
            c[0, base : base + 256] = 1.0
            c[1, base : base + 256] = beta_u[:256]
            c[2, base + 256 : base + 512] = 1.0
            c[3, base + 256 : base + 512] = beta_u[256:]
        return c

    consts_bf = np.ascontiguousarray(wT.astype(ml_dtypes.bfloat16))
    return consts_for(False), consts_for(True), consts_bf


def kernel(x, weight, bias, ln_gamma, ln_beta):
    x = np.ascontiguousarray(x, dtype=np.float32)
    weight = np.asarray(weight, dtype=np.float32)
    bias = np.asarray(bias, dtype=np.float32)
    ln_gamma = np.asarray(ln_gamma, dtype=np.float32)
    ln_beta = np.asarray(ln_beta, dtype=np.float32)

    consts_even, consts_odd, consts_bf = _host_prep(weight, bias, ln_beta)

    bias_uniform = bool(np.all(bias == bias.flat[0]))
    general = not (
        np.all(ln_gamma == 1.0) and np.all(ln_beta == 0.0) and bias_uniform
    )
    bias_val = float(bias.flat[0]) if bias_uniform else 0.0
    key = (general, bias_val)
    if key not in _NC_CACHE:
        _NC_CACHE[key] = _build_nc(general, bias_val)
    nc = _NC_CACHE[key]

    half = N // 2
    nblk = BLK_PER_CORE
    res_np_dt = np.float32 if general else ml_dtypes.bfloat16
    gate_f8 = np.ascontiguousarray(x[:, :, DOUT:]).astype(ml_dtypes.float8_e4m3)

    def to_pmajor(a, nb):
        # [nb*W, DOUT] -> [W, nb*DOUT] (partition-major for flat 2D DMAs)
        return np.ascontiguousarray(
            a.reshape(nb, W, DOUT).transpose(1, 0, 2).reshape(W, nb * DOUT)
        )

    in_maps = []
    for k in range(NCORES):
        bk, hk = k // 2, k % 2
        res_sh = to_pmajor(
            x[bk, hk * half : (hk + 1) * half, :DOUT].astype(res_np_dt), nblk
        )
        if hk == 0:
            halo = np.zeros((W, DOUT), dtype=ml_dtypes.float8_e4m3)
        else:
            halo = gate_f8[bk, half - W : half]
        gate_sh = to_pmajor(
            np.concatenate(
                [halo, gate_f8[bk, hk * half : (hk + 1) * half]], axis=0
            ),
            nblk + 1,
        )
        m = {
            "res_sh": res_sh,
            "gate_sh": gate_sh,
            "consts4": consts_odd if hk == 1 else consts_even,
            "consts_bf": consts_bf,
        }
        if general:
            m["gamma"] = ln_gamma
        in_maps.append(m)

    global _last_in_maps
    _last_in_maps = in_maps

    res = run_bass_kernel_spmd(nc, in_maps, list(range(NCORES)))

    out = np.empty((B, N, DOUT), dtype=np.float32)
    for k in range(NCORES):
        bk, hk = k // 2, k % 2
        o = res.results[k]["out"]  # [W, nblk*DOUT] partition-major
        o = o.reshape(W, nblk, DOUT).transpose(1, 0, 2).reshape(half, DOUT)
        out[bk, hk * half : (hk + 1) * half] = o.astype(np.float32)
    return out


# revision 34
# speedup vs baseline: 1.0007x; 1.0007x over previous
"""CausalLocalSGU Trainium2 kernel.

Reference computation (per batch b):
  split x[b] channels -> res (first 1024), gate_in (last 1024)
  per 128-token window block j: z_j = LayerNorm(gate_in_j) * gamma + beta
  gate_out_j[m, c] = sum_n W[h(c), m, n] * [z_{j-1}; z_j][n, c] + bias[h(c), m]
      (W masked causally: keep [m, n] where n <= m + 128; z_{-1} = 0)
  out_j = gate_out_j * res_j

Sharding: 8 cores; core k handles batch k//2, token half k%2 (2048 tokens =
16 window blocks) plus a one-block halo on the left (zeros for even cores).
Halo LNs are recomputed locally -> no collectives.

Precision: gate half is cast to fp8-e4m3 on the host (it only feeds the
~7e-5-magnitude SGU einsum term; weights ~1e-5).  res and out travel as
bf16 (~0.2% rel err; tolerance is 2e-2); the host upcasts the output to
fp32.  LN stats are estimated from every other channel (512 samples; the
~2% rstd estimate error perturbs the output by ~1e-6 relative, far below
the fp8-gate noise).  This cuts per-core HBM traffic from 19.3 MB to
10.6 MB and halves the DVE bn_stats cost.

Structure: each core runs its 16 blocks as TWO independent 8-block chains
(the second chain re-LNs gate block 8 as its halo).  The per-block
dependency chain (stats->aggr->rstd->negmu->znorm->matmuls->drain->mult->
store) is ~6 us deep; two interleaved chains keep every engine fed and
halve pipeline fill/drain.  Per block [128,1024]:
  DVE:  bn_stats (subsampled, 1x-only op) + bn_aggr + negmu + one
        macro-wide combine multiply per 4 blocks (t_t bf16 2x mode),
        issued one step late so it never waits in the in-order stream
  ACT:  rstd (Abs_reciprocal_sqrt) + psum drains (+bias imm, fp32->bf16)
  Gp:   all LN-normalizes (fp8-in tensor_scalar mult/add w/ dense [W,1]
        scalar tiles -- the only fast Q7 form; Pool has no PSUM access)
  PE:   8 matmuls + 8 ldweights per block, bf16 (z bf16 x wt bf16)
DMA: all loads and stores ride the sync HWDGE ring as flat 2D copies of
host pre-interleaved partition-major [W, blocks*DOUT] arrays (constant
descriptor issue cost); store issue on gpsimd would convoy the znorms.

Fast path requires gamma == ones, beta == zeros and a uniform bias;
anything else compiles the general variant (full-width stats, fp32
extras matmul carrying bias + S*beta, explicit gamma multiply, fp32
res/out, single chain).
"""

import ml_dtypes
import numpy as np

import concourse.bacc as bacc
import concourse.bass as bass
import concourse.tile as tile
from concourse import mybir
from concourse.bass_utils import run_bass_kernel_spmd

F32 = mybir.dt.float32
BF16 = mybir.dt.bfloat16
FP8 = mybir.dt.float8e4

HEADS = 4
W = 128            # window
DIM = 2048
DOUT = 1024        # dim // 2
DHEAD = DOUT // HEADS  # 256
B = 4
N = 4096
NCORES = 8
BLK_PER_CORE = (N // 2) // W   # 16
MACRO = 4          # window blocks per output store batch
CHAIN = 8          # blocks per independent pipeline chain
LN_EPS = 1e-5

# fp32 consts layout ([4, 1536]): K=4 extras matmul operands (general path).
_EXR0 = 0           # [4, 256]: lhsT, halves 0,1 (S = S_full)
_EXF0 = 256         # [4, 256]: lhsT, halves 0,1 (S = S_first)
_RHSX0 = 512        # [4, 1024]: rhs for half 0 then half 1
_CONSTS_COLS = 1536

_NC_CACHE: dict = {}
_last_in_maps: list = []


def _build_nc(general: bool, bias_val: float = 1.0) -> bass.Bass:
    nc = bacc.Bacc(
        trn_type="TRN2",
        target_bir_lowering=False,
        debug=False,
        num_devices=NCORES,
    )
    nblk = BLK_PER_CORE  # output blocks per core; +1 halo block for gate
    res_dt = F32 if general else BF16
    # partition-major layouts ([W, blocks*DOUT], host pre-interleaved):
    # every DMA is a flat contiguous 2D copy
    res_sh = nc.dram_tensor("res_sh", [W, nblk * DOUT], res_dt,
                            kind="ExternalInput").ap()
    gate_sh = nc.dram_tensor(
        "gate_sh", [W, (nblk + 1) * DOUT], FP8, kind="ExternalInput"
    ).ap()
    consts4 = nc.dram_tensor(
        "consts4", [4, _CONSTS_COLS], F32, kind="ExternalInput"
    ).ap()
    consts_bf = nc.dram_tensor(
        "consts_bf", [W, 2 * HEADS * W], BF16, kind="ExternalInput"
    ).ap()
    if general:
        gamma = nc.dram_tensor("gamma", [DOUT], F32, kind="ExternalInput").ap()
    out = nc.dram_tensor("out", [W, nblk * DOUT], res_dt,
                         kind="ExternalOutput").ap()

    ident = mybir.ActivationFunctionType.Identity
    alu = mybir.AluOpType

    with tile.TileContext(nc) as tc:
        with (
            tc.tile_pool(name="singles", bufs=1) as singles,
            tc.tile_pool(name="gpool", bufs=1) as gpool,
            tc.tile_pool(name="rpool", bufs=4) as rpool,
            tc.tile_pool(name="opool", bufs=4) as opool,
            tc.tile_pool(name="zpool", bufs=8) as zpool,
            tc.tile_pool(name="cpool", bufs=4) as cpool,
            tc.tile_pool(name="spool", bufs=16) as spool,
            tc.tile_pool(name="ppool", bufs=4, space="PSUM") as ppool,
        ):
            consts4_t = singles.tile([4, _CONSTS_COLS], F32)
            wt_t = singles.tile([W, 2 * HEADS * W], BF16)
            eps_t = singles.tile([128, 1], F32)
            nc.vector.memset(eps_t, LN_EPS)
            if general:
                gamma_t = singles.tile([128, DOUT], F32)

            # halo block load first (smallest, unblocks chain A)
            gate0 = gpool.tile([W, DOUT], FP8, tag="gate0")
            nc.sync.dma_start(out=gate0, in_=gate_sh[:, 0:DOUT])
            if general:
                nc.gpsimd.dma_start(
                    out=gamma_t,
                    in_=bass.AP(
                        tensor=gamma.tensor,
                        offset=gamma.offset,
                        ap=[[0, 128]] + list(gamma.ap),
                    ),
                )
            exr_t = consts4_t[:, _EXR0 : _EXR0 + 2 * W]
            exf_t = consts4_t[:, _EXF0 : _EXF0 + 2 * W]
            rhsx_t = consts4_t[:, _RHSX0 : _RHSX0 + DOUT]

            # sync-ring order: small early gate chunks unblock both
            # chains' LN heads, then bulk; res macros land just in time
            nmac = nblk // MACRO
            gseg = []
            r4s = [None] * nmac

            def load_g(lo, n):
                gt = gpool.tile([W, n * DOUT], FP8, tag=f"g{lo}")
                nc.sync.dma_start(
                    out=gt, in_=gate_sh[:, lo * DOUT : (lo + n) * DOUT]
                )
                gseg.append((lo, n, gt))

            def load_r4(m):
                r4 = rpool.tile([W, MACRO * DOUT], res_dt, tag="r4")
                nc.sync.dma_start(
                    out=r4,
                    in_=res_sh[:, m * MACRO * DOUT : (m + 1) * MACRO * DOUT],
                )
                r4s[m] = r4

            load_g(1, 2)       # chain A head
            load_g(8, 2)       # chain B head (halo 8 + block 9)
            nc.sync.dma_start(out=wt_t, in_=consts_bf)
            load_g(3, 5)       # chain A bulk
            load_g(10, 7)      # chain B bulk (both chains eat gate early)
            nc.sync.dma_start(out=consts4_t, in_=consts4)
            load_r4(0)
            load_r4(2)
            load_r4(1)
            load_r4(3)

            def gate_ap(gb):
                if gb == 0:
                    return gate0
                for lo, n, gt in gseg:
                    if lo <= gb < lo + n:
                        return gt[:, (gb - lo) * DOUT : (gb - lo + 1) * DOUT]
                raise AssertionError(gb)

            def res_ap(blk, n=1):
                lo = (blk % MACRO) * DOUT
                return r4s[blk // MACRO][:, lo : lo + n * DOUT]

            def ln_stats(gate):
                """stage 1: bn stats + rstd request (DVE + ACT)."""
                if general:
                    stats = spool.tile([W, 2, 6], F32, tag="stats")
                    nc.vector.bn_stats(out=stats[:, 0], in_=gate[:, :512])
                    nc.vector.bn_stats(out=stats[:, 1], in_=gate[:, 512:])
                else:
                    stats = spool.tile([W, 6], F32, tag="stats")
                    nc.vector.bn_stats(out=stats, in_=gate[:, 0:DOUT:4])
                mv = spool.tile([W, 2], F32, tag="mv")
                nc.vector.bn_aggr(out=mv, in_=stats)
                rstd = spool.tile([W, 1], F32, tag="rstd")
                nc.scalar.activation(
                    out=rstd,
                    in_=mv[:, 1:2],
                    func=mybir.ActivationFunctionType.Abs_reciprocal_sqrt,
                    bias=eps_t,
                )
                return mv, rstd

            def ln_norm(gate, mv, rstd):
                """stage 2: normalize into a bf16 z tile (GpSimd)."""
                negmu = spool.tile([W, 1], F32, tag="negmu")
                nc.vector.tensor_scalar(
                    out=negmu,
                    in0=mv[:, 0:1],
                    scalar1=rstd,
                    scalar2=-1.0,
                    op0=alu.mult,
                    op1=alu.mult,
                )
                z = zpool.tile([W, DOUT], BF16, tag="z")
                if not general:
                    nc.gpsimd.tensor_scalar(
                        out=z, in0=gate, scalar1=rstd, scalar2=negmu,
                        op0=alu.mult, op1=alu.add,
                    )
                    return z
                nc.scalar.activation(
                    out=z, in_=gate, func=ident, bias=negmu, scale=rstd
                )
                nc.vector.tensor_mul(z, z, gamma_t)
                return z

            def store_macro(mq, o4, last):
                if not last:
                    nc.sync.dma_start(
                        out=out[:, mq * MACRO * DOUT : (mq + 1) * MACRO * DOUT],
                        in_=o4,
                    )
                    return
                # chain-final macro ships pair + singles: short store tail
                for lo, n in [(0, 2), (2, 1), (3, 1)]:
                    nc.sync.dma_start(
                        out=out[:, (mq * MACRO + lo) * DOUT
                                : (mq * MACRO + lo + n) * DOUT],
                        in_=o4[:, lo * DOUT : (lo + n) * DOUT],
                    )

            if general:
                # single-chain reference-shaped loop (correctness fallback)
                lnq = [ln_stats(gate_ap(0)), ln_stats(gate_ap(1))]
                z_prev = None
                o4 = None
                for gb in range(nblk + 1):
                    if gb + 2 <= nblk:
                        lnq.append(ln_stats(gate_ap(gb + 2)))
                    blk = gb - 1
                    if blk >= 0 and blk % MACRO == 0:
                        o4 = opool.tile([W, MACRO * DOUT], res_dt, tag="o4")
                    mv_c, rstd_c = lnq.pop(0)
                    z = ln_norm(gate_ap(gb), mv_c, rstd_c)
                    if blk >= 0:
                        s = blk % MACRO
                        psum = ppool.tile([W, DOUT], F32, tag="psum")
                        ex_t = exf_t if blk == 0 else exr_t
                        for u in range(2):
                            nc.tensor.matmul(
                                psum[:, u * 512 : (u + 1) * 512],
                                ex_t[:, u * W : (u + 1) * W],
                                rhsx_t[:, u * 512 : (u + 1) * 512],
                                start=True,
                                stop=False,
                            )
                            for h in (2 * u, 2 * u + 1):
                                ps = psum[:, h * DHEAD : (h + 1) * DHEAD]
                                nc.tensor.matmul(
                                    ps,
                                    wt_t[:, (2 * h) * W : (2 * h + 1) * W],
                                    z_prev[:, h * DHEAD : (h + 1) * DHEAD],
                                    start=False,
                                    stop=False,
                                )
                                nc.tensor.matmul(
                                    ps,
                                    wt_t[:, (2 * h + 1) * W : (2 * h + 2) * W],
                                    z[:, h * DHEAD : (h + 1) * DHEAD],
                                    start=False,
                                    stop=(h == 2 * u + 1),
                                )
                        nc.vector.tensor_mul(
                            o4[:, s * DOUT : (s + 1) * DOUT], psum, res_ap(blk)
                        )
                        if s == MACRO - 1:
                            store_macro(blk // MACRO, o4, last=False)
                    z_prev = z
            else:
                # two interleaved independent chains (c = 0, 1); chain c
                # covers output blocks 8c..8c+7, LN steps j = 0..8 map to
                # gate blocks 8c + j (j = 0 is the chain halo)
                LA = 2  # per-chain stats lookahead (4 in absolute ops)
                st = [
                    {"lnq": [], "z_prev": None, "o4": None, "c16": None}
                    for _ in range(2)
                ]
                for j in range(LA):
                    for c in range(2):
                        st[c]["lnq"].append(ln_stats(gate_ap(CHAIN * c + j)))
                pend = []      # delayed macro mult+store work

                def do_piece(mq, m_c16, m_o4, lo, n):
                    sl = slice(lo * DOUT, (lo + n) * DOUT)
                    nc.vector.tensor_mul(
                        m_o4[:, sl], m_c16[:, sl], r4s[mq][:, sl]
                    )
                    nc.sync.dma_start(
                        out=out[:, (mq * MACRO + lo) * DOUT
                                : (mq * MACRO + lo + n) * DOUT],
                        in_=m_o4[:, sl],
                    )
                for j in range(CHAIN + 1):
                    for c in range(2):
                        S = st[c]
                        # flush a completed macro's multiply+store one slot
                        # late: its c16 inputs are surely drained, so the
                        # wide DVE t_t never stalls the in-order stream
                        while pend:
                            do_piece(*pend.pop(0))
                        if j + LA <= CHAIN:
                            S["lnq"].append(
                                ln_stats(gate_ap(CHAIN * c + j + LA))
                            )
                        blk = CHAIN * c + j - 1
                        mv_c, rstd_c = S["lnq"].pop(0)
                        psum = None
                        if j >= 1:
                            if (j - 1) % MACRO == 0:
                                S["o4"] = opool.tile(
                                    [W, MACRO * DOUT], res_dt, tag="o4",
                                    name=f"o4_{c}_{j}",
                                )
                                S["c16"] = cpool.tile(
                                    [W, MACRO * DOUT], BF16, tag="c16",
                                    name=f"c16_{c}_{j}",
                                )
                            # prev-window matmuls only need z_prev: the PE
                            # works while this block's znorm is in flight
                            psum = ppool.tile([W, DOUT], F32, tag="psum")
                            for h in range(HEADS):
                                nc.tensor.matmul(
                                    psum[:, h * DHEAD : (h + 1) * DHEAD],
                                    wt_t[:, (2 * h) * W : (2 * h + 1) * W],
                                    S["z_prev"][:, h * DHEAD : (h + 1) * DHEAD],
                                    start=True,
                                    stop=False,
                                )
                        z = ln_norm(gate_ap(CHAIN * c + j), mv_c, rstd_c)
                        if j >= 1:
                            for h in range(HEADS):
                                nc.tensor.matmul(
                                    psum[:, h * DHEAD : (h + 1) * DHEAD],
                                    wt_t[:, (2 * h + 1) * W : (2 * h + 2) * W],
                                    z[:, h * DHEAD : (h + 1) * DHEAD],
                                    start=False,
                                    stop=True,
                                )
                            s = (j - 1) % MACRO
                            nc.scalar.activation(
                                out=S["c16"][:, s * DOUT : (s + 1) * DOUT],
                                in_=psum, func=ident,
                                bias=float(bias_val), scale=1.0,
                            )
                            if j > CHAIN - MACRO:
                                # chain-final macro streams out in pieces
                                # as soon as each drain lands
                                if s == 1:
                                    pend.append((blk // MACRO, S["c16"],
                                                 S["o4"], 0, 2))
                                elif s >= 2:
                                    pend.append((blk // MACRO, S["c16"],
                                                 S["o4"], s, 1))
                            elif s == MACRO - 1:
                                pend.append((
                                    blk // MACRO, S["c16"], S["o4"], 0, MACRO,
                                ))
                        S["z_prev"] = z
                while pend:
                    do_piece(*pend.pop(0))
    if not nc.is_finalized():
        nc.finalize()
    return nc


def _host_prep(weight, bias, ln_beta):
    j = np.arange(2 * W)[None, :]
    i_ = np.arange(W)[:, None]
    mask = (j <= i_ + W).astype(np.float32)          # [W, 2W]
    wm = weight * mask[None]                         # [H, W, 2W]
    wT = np.zeros((W, 2 * HEADS, W), dtype=np.float32)
    for h in range(HEADS):
        wT[:, 2 * h] = wm[h, :, :W].T                # A_h: prev-window cols
        wT[:, 2 * h + 1] = wm[h, :, W:].T            # B_h: current-window cols
    wT = wT.reshape(W, 2 * HEADS * W)

    s_full = wm.sum(-1)                              # [H, W]
    s_first = wm[:, :, W:].sum(-1)

    def consts_for(first_has_prev: bool):
        c = np.zeros((4, _CONSTS_COLS), dtype=np.float32)
        sf = s_full if first_has_prev else s_first
        for u in range(2):
            # lhsT rows: bias[2u], S[2u], bias[2u+1], S[2u+1]
            c[0, _EXR0 + u * W : _EXR0 + (u + 1) * W] = bias[2 * u]
            c[1, _EXR0 + u * W : _EXR0 + (u + 1) * W] = s_full[2 * u]
            c[2, _EXR0 + u * W : _EXR0 + (u + 1) * W] = bias[2 * u + 1]
            c[3, _EXR0 + u * W : _EXR0 + (u + 1) * W] = s_full[2 * u + 1]
            c[0, _EXF0 + u * W : _EXF0 + (u + 1) * W] = bias[2 * u]
            c[1, _EXF0 + u * W : _EXF0 + (u + 1) * W] = sf[2 * u]
            c[2, _EXF0 + u * W : _EXF0 + (u + 1) * W] = bias[2 * u + 1]
            c[3, _EXF0 + u * W : _EXF0 + (u + 1) * W] = sf[2 * u + 1]
            # rhs rows: ind[2u], beta*ind[2u], ind[2u+1], beta*ind[2u+1]
            base = _RHSX0 + u * 512
            beta_u = ln_beta[u * 512 : (u + 1) * 512]
            c[0, base : base + 256] = 1.0
            c[1, base : base + 256] = beta_u[:256]
            c[2, base + 256 : base + 512] = 1.0
            c[3, base + 256 : base + 512] = beta_u[256:]
        return c

    consts_bf = np.ascontiguousarray(wT.astype(ml_dtypes.bfloat16))
    return consts_for(False), consts_for(True), consts_bf


def kernel(x, weight, bias, ln_gamma, ln_beta):
    x = np.ascontiguousarray(x, dtype=np.float32)
    weight = np.asarray(weight, dtype=np.float32)
    bias = np.asarray(bias, dtype=np.float32)
    ln_gamma = np.asarray(ln_gamma, dtype=np.float32)
    ln_beta = np.asarray(ln_beta, dtype=np.float32)

    consts_even, consts_odd, consts_bf = _host_prep(weight, bias, ln_beta)

    bias_uniform = bool(np.all(bias == bias.flat[0]))
    general = not (
        np.all(ln_gamma == 1.0) and np.all(ln_beta == 0.0) and bias_uniform
    )
    bias_val = float(bias.flat[0]) if bias_uniform else 0.0
    key = (general, bias_val)
    if key not in _NC_CACHE:
        _NC_CACHE[key] = _build_nc(general, bias_val)
    nc = _NC_CACHE[key]

    half = N // 2
    nblk = BLK_PER_CORE
    res_np_dt = np.float32 if general else ml_dtypes.bfloat16
    gate_f8 = np.ascontiguousarray(x[:, :, DOUT:]).astype(ml_dtypes.float8_e4m3)

    def to_pmajor(a, nb):
        # [nb*W, DOUT] -> [W, nb*DOUT] (partition-major for flat 2D DMAs)
        return np.ascontiguousarray(
            a.reshape(nb, W, DOUT).transpose(1, 0, 2).reshape(W, nb * DOUT)
        )

    in_maps = []
    for k in range(NCORES):
        bk, hk = k // 2, k % 2
        res_sh = to_pmajor(
            x[bk, hk * half : (hk + 1) * half, :DOUT].astype(res_np_dt), nblk
        )
        if hk == 0:
            halo = np.zeros((W, DOUT), dtype=ml_dtypes.float8_e4m3)
        else:
            halo = gate_f8[bk, half - W : half]
        gate_sh = to_pmajor(
            np.concatenate(
                [halo, gate_f8[bk, hk * half : (hk + 1) * half]], axis=0
            ),
            nblk + 1,
        )
        m = {
            "res_sh": res_sh,
            "gate_sh": gate_sh,
            "consts4": consts_odd if hk == 1 else consts_even,
            "consts_bf": consts_bf,
        }
        if general:
            m["gamma"] = ln_gamma
        in_maps.append(m)

    global _last_in_maps
    _last_in_maps = in_maps

    res = run_bass_kernel_spmd(nc, in_maps, list(range(NCORES)))

    out = np.empty((B, N, DOUT), dtype=np.float32)
    for k in range(NCORES):
        bk, hk = k // 2, k % 2
        o = res.results[k]["out"]  # [W, nblk*DOUT] partition-major
        o = o.reshape(W, nblk, DOUT).transpose(1, 0, 2).reshape(half, DOUT)
        out[bk, hk * half : (hk + 1) * half] = o.astype(np.float32)
    return out


# revision 35
# speedup vs baseline: 1.0951x; 1.0943x over previous
"""CausalLocalSGU Trainium2 kernel.

Reference computation (per batch b):
  split x[b] channels -> res (first 1024), gate_in (last 1024)
  per 128-token window block j: z_j = LayerNorm(gate_in_j) * gamma + beta
  gate_out_j[m, c] = sum_n W[h(c), m, n] * [z_{j-1}; z_j][n, c] + bias[h(c), m]
      (W masked causally: keep [m, n] where n <= m + 128; z_{-1} = 0)
  out_j = gate_out_j * res_j

Sharding: 8 cores; core k handles batch k//2, token half k%2 (2048 tokens =
16 window blocks) plus a one-block halo on the left (zeros for even cores).
Halo LNs are recomputed locally -> no collectives.

Precision: gate half is cast to fp8-e4m3 on the host (it only feeds the
~7e-5-magnitude SGU einsum term; weights ~1e-5).  res and out travel as
bf16 (~0.2% rel err; tolerance is 2e-2); the host upcasts the output to
fp32.  LN stats are estimated from every other channel (512 samples; the
~2% rstd estimate error perturbs the output by ~1e-6 relative, far below
the fp8-gate noise).  This cuts per-core HBM traffic from 19.3 MB to
10.6 MB and halves the DVE bn_stats cost.

Structure: each core runs its 16 blocks as TWO independent 8-block chains
(the second chain re-LNs gate block 8 as its halo).  The per-block
dependency chain (stats->aggr->rstd->negmu->znorm->matmuls->drain->mult->
store) is ~6 us deep; two interleaved chains keep every engine fed and
halve pipeline fill/drain.  Per block [128,1024]:
  DVE:  bn_stats (subsampled, 1x-only op) + bn_aggr + negmu + one
        macro-wide combine multiply per 4 blocks (t_t bf16 2x mode),
        issued one step late so it never waits in the in-order stream
  ACT:  rstd (Abs_reciprocal_sqrt) + psum drains (+bias imm, fp32->bf16)
  Gp:   all LN-normalizes (fp8-in tensor_scalar mult/add w/ dense [W,1]
        scalar tiles -- the only fast Q7 form; Pool has no PSUM access)
  PE:   8 matmuls + 8 ldweights per block, bf16 (z bf16 x wt bf16)
DMA: all loads and stores ride the sync HWDGE ring as flat 2D copies of
host pre-interleaved partition-major [W, blocks*DOUT] arrays (constant
descriptor issue cost); store issue on gpsimd would convoy the znorms.

Fast path requires gamma == ones, beta == zeros and a uniform bias;
anything else compiles the general variant (full-width stats, fp32
extras matmul carrying bias + S*beta, explicit gamma multiply, fp32
res/out, single chain).
"""

import ml_dtypes
import numpy as np

import concourse.bacc as bacc
import concourse.bass as bass
import concourse.tile as tile
from concourse import mybir
from concourse.bass_utils import run_bass_kernel_spmd

F32 = mybir.dt.float32
BF16 = mybir.dt.bfloat16
FP8 = mybir.dt.float8e4

HEADS = 4
W = 128            # window
DIM = 2048
DOUT = 1024        # dim // 2
DHEAD = DOUT // HEADS  # 256
B = 4
N = 4096
NCORES = 8
BLK_PER_CORE = (N // 2) // W   # 16
MACRO = 2          # window blocks per output store batch
CHAIN = 8          # blocks per independent pipeline chain
LN_EPS = 1e-5

# fp32 consts layout ([4, 1536]): K=4 extras matmul operands (general path).
_EXR0 = 0           # [4, 256]: lhsT, halves 0,1 (S = S_full)
_EXF0 = 256         # [4, 256]: lhsT, halves 0,1 (S = S_first)
_RHSX0 = 512        # [4, 1024]: rhs for half 0 then half 1
_CONSTS_COLS = 1536

_NC_CACHE: dict = {}
_last_in_maps: list = []


def _build_nc(general: bool, bias_val: float = 1.0) -> bass.Bass:
    nc = bacc.Bacc(
        trn_type="TRN2",
        target_bir_lowering=False,
        debug=False,
        num_devices=NCORES,
    )
    nblk = BLK_PER_CORE  # output blocks per core; +1 halo block for gate
    res_dt = F32 if general else BF16
    # partition-major layouts ([W, blocks*DOUT], host pre-interleaved):
    # every DMA is a flat contiguous 2D copy
    res_sh = nc.dram_tensor("res_sh", [W, nblk * DOUT], res_dt,
                            kind="ExternalInput").ap()
    gate_sh = nc.dram_tensor(
        "gate_sh", [W, (nblk + 1) * DOUT], FP8, kind="ExternalInput"
    ).ap()
    consts4 = nc.dram_tensor(
        "consts4", [4, _CONSTS_COLS], F32, kind="ExternalInput"
    ).ap()
    consts_bf = nc.dram_tensor(
        "consts_bf", [W, 2 * HEADS * W], BF16, kind="ExternalInput"
    ).ap()
    if general:
        gamma = nc.dram_tensor("gamma", [DOUT], F32, kind="ExternalInput").ap()
    out = nc.dram_tensor("out", [W, nblk * DOUT], res_dt,
                         kind="ExternalOutput").ap()

    ident = mybir.ActivationFunctionType.Identity
    alu = mybir.AluOpType

    with tile.TileContext(nc) as tc:
        with (
            tc.tile_pool(name="singles", bufs=1) as singles,
            tc.tile_pool(name="gpool", bufs=1) as gpool,
            tc.tile_pool(name="rpool", bufs=8) as rpool,
            tc.tile_pool(name="opool", bufs=6) as opool,
            tc.tile_pool(name="zpool", bufs=8) as zpool,
            tc.tile_pool(name="cpool", bufs=6) as cpool,
            tc.tile_pool(name="spool", bufs=16) as spool,
            tc.tile_pool(name="ppool", bufs=4, space="PSUM") as ppool,
        ):
            consts4_t = singles.tile([4, _CONSTS_COLS], F32)
            wt_t = singles.tile([W, 2 * HEADS * W], BF16)
            eps_t = singles.tile([128, 1], F32)
            nc.vector.memset(eps_t, LN_EPS)
            if general:
                gamma_t = singles.tile([128, DOUT], F32)

            # halo block load first (smallest, unblocks chain A)
            gate0 = gpool.tile([W, DOUT], FP8, tag="gate0")
            nc.sync.dma_start(out=gate0, in_=gate_sh[:, 0:DOUT])
            if general:
                nc.gpsimd.dma_start(
                    out=gamma_t,
                    in_=bass.AP(
                        tensor=gamma.tensor,
                        offset=gamma.offset,
                        ap=[[0, 128]] + list(gamma.ap),
                    ),
                )
            exr_t = consts4_t[:, _EXR0 : _EXR0 + 2 * W]
            exf_t = consts4_t[:, _EXF0 : _EXF0 + 2 * W]
            rhsx_t = consts4_t[:, _RHSX0 : _RHSX0 + DOUT]

            # sync-ring order: small early gate chunks unblock both
            # chains' LN heads, then bulk; res macros land just in time
            nmac = nblk // MACRO
            gseg = []
            r4s = [None] * nmac

            def load_g(lo, n):
                gt = gpool.tile([W, n * DOUT], FP8, tag=f"g{lo}")
                nc.sync.dma_start(
                    out=gt, in_=gate_sh[:, lo * DOUT : (lo + n) * DOUT]
                )
                gseg.append((lo, n, gt))

            def load_r4(m):
                r4 = rpool.tile([W, MACRO * DOUT], res_dt, tag="r4")
                nc.sync.dma_start(
                    out=r4,
                    in_=res_sh[:, m * MACRO * DOUT : (m + 1) * MACRO * DOUT],
                )
                r4s[m] = r4

            load_g(1, 2)       # chain A head
            load_g(8, 2)       # chain B head (halo 8 + block 9)
            nc.sync.dma_start(out=wt_t, in_=consts_bf)
            load_g(3, 5)       # chain A bulk
            load_g(10, 7)      # chain B bulk (both chains eat gate early)
            nc.sync.dma_start(out=consts4_t, in_=consts4)
            for m in (0, 4, 1, 5, 2, 6, 3, 7):
                load_r4(m)

            def gate_ap(gb):
                if gb == 0:
                    return gate0
                for lo, n, gt in gseg:
                    if lo <= gb < lo + n:
                        return gt[:, (gb - lo) * DOUT : (gb - lo + 1) * DOUT]
                raise AssertionError(gb)

            def res_ap(blk, n=1):
                lo = (blk % MACRO) * DOUT
                return r4s[blk // MACRO][:, lo : lo + n * DOUT]

            def ln_stats(gate):
                """stage 1: bn stats + rstd request (DVE + ACT)."""
                if general:
                    stats = spool.tile([W, 2, 6], F32, tag="stats")
                    nc.vector.bn_stats(out=stats[:, 0], in_=gate[:, :512])
                    nc.vector.bn_stats(out=stats[:, 1], in_=gate[:, 512:])
                else:
                    stats = spool.tile([W, 6], F32, tag="stats")
                    nc.vector.bn_stats(out=stats, in_=gate[:, 0:DOUT:4])
                mv = spool.tile([W, 2], F32, tag="mv")
                nc.vector.bn_aggr(out=mv, in_=stats)
                rstd = spool.tile([W, 1], F32, tag="rstd")
                nc.scalar.activation(
                    out=rstd,
                    in_=mv[:, 1:2],
                    func=mybir.ActivationFunctionType.Abs_reciprocal_sqrt,
                    bias=eps_t,
                )
                return mv, rstd

            def ln_norm(gate, mv, rstd):
                """stage 2: normalize into a bf16 z tile (GpSimd)."""
                negmu = spool.tile([W, 1], F32, tag="negmu")
                nc.vector.tensor_scalar(
                    out=negmu,
                    in0=mv[:, 0:1],
                    scalar1=rstd,
                    scalar2=-1.0,
                    op0=alu.mult,
                    op1=alu.mult,
                )
                z = zpool.tile([W, DOUT], BF16, tag="z")
                if not general:
                    nc.gpsimd.tensor_scalar(
                        out=z, in0=gate, scalar1=rstd, scalar2=negmu,
                        op0=alu.mult, op1=alu.add,
                    )
                    return z
                nc.scalar.activation(
                    out=z, in_=gate, func=ident, bias=negmu, scale=rstd
                )
                nc.vector.tensor_mul(z, z, gamma_t)
                return z

            def store_macro(mq, o4, last):
                if not last:
                    nc.sync.dma_start(
                        out=out[:, mq * MACRO * DOUT : (mq + 1) * MACRO * DOUT],
                        in_=o4,
                    )
                    return
                # chain-final macro ships pair + singles: short store tail
                for lo, n in [(0, 2), (2, 1), (3, 1)]:
                    nc.sync.dma_start(
                        out=out[:, (mq * MACRO + lo) * DOUT
                                : (mq * MACRO + lo + n) * DOUT],
                        in_=o4[:, lo * DOUT : (lo + n) * DOUT],
                    )

            if general:
                # single-chain reference-shaped loop (correctness fallback)
                lnq = [ln_stats(gate_ap(0)), ln_stats(gate_ap(1))]
                z_prev = None
                o4 = None
                for gb in range(nblk + 1):
                    if gb + 2 <= nblk:
                        lnq.append(ln_stats(gate_ap(gb + 2)))
                    blk = gb - 1
                    if blk >= 0 and blk % MACRO == 0:
                        o4 = opool.tile([W, MACRO * DOUT], res_dt, tag="o4")
                    mv_c, rstd_c = lnq.pop(0)
                    z = ln_norm(gate_ap(gb), mv_c, rstd_c)
                    if blk >= 0:
                        s = blk % MACRO
                        psum = ppool.tile([W, DOUT], F32, tag="psum")
                        ex_t = exf_t if blk == 0 else exr_t
                        for u in range(2):
                            nc.tensor.matmul(
                                psum[:, u * 512 : (u + 1) * 512],
                                ex_t[:, u * W : (u + 1) * W],
                                rhsx_t[:, u * 512 : (u + 1) * 512],
                                start=True,
                                stop=False,
                            )
                            for h in (2 * u, 2 * u + 1):
                                ps = psum[:, h * DHEAD : (h + 1) * DHEAD]
                                nc.tensor.matmul(
                                    ps,
                                    wt_t[:, (2 * h) * W : (2 * h + 1) * W],
                                    z_prev[:, h * DHEAD : (h + 1) * DHEAD],
                                    start=False,
                                    stop=False,
                                )
                                nc.tensor.matmul(
                                    ps,
                                    wt_t[:, (2 * h + 1) * W : (2 * h + 2) * W],
                                    z[:, h * DHEAD : (h + 1) * DHEAD],
                                    start=False,
                                    stop=(h == 2 * u + 1),
                                )
                        nc.vector.tensor_mul(
                            o4[:, s * DOUT : (s + 1) * DOUT], psum, res_ap(blk)
                        )
                        if s == MACRO - 1:
                            store_macro(blk // MACRO, o4, last=False)
                    z_prev = z
            else:
                # two interleaved independent chains (c = 0, 1); chain c
                # covers output blocks 8c..8c+7, LN steps j = 0..8 map to
                # gate blocks 8c + j (j = 0 is the chain halo)
                LA = 2  # per-chain stats lookahead (4 in absolute ops)
                st = [
                    {"lnq": [], "z_prev": None, "o4": None, "c16": None}
                    for _ in range(2)
                ]
                for j in range(LA):
                    for c in range(2):
                        st[c]["lnq"].append(ln_stats(gate_ap(CHAIN * c + j)))
                pend = []      # delayed macro mult+store work

                def do_piece(mq, m_c16, m_o4, lo, n):
                    sl = slice(lo * DOUT, (lo + n) * DOUT)
                    nc.vector.tensor_mul(
                        m_o4[:, sl], m_c16[:, sl], r4s[mq][:, sl]
                    )
                    nc.sync.dma_start(
                        out=out[:, (mq * MACRO + lo) * DOUT
                                : (mq * MACRO + lo + n) * DOUT],
                        in_=m_o4[:, sl],
                    )
                for j in range(CHAIN + 1):
                    for c in range(2):
                        S = st[c]
                        # flush a completed macro's multiply+store one slot
                        # late: its c16 inputs are surely drained, so the
                        # wide DVE t_t never stalls the in-order stream
                        while pend:
                            do_piece(*pend.pop(0))
                        if j + LA <= CHAIN:
                            S["lnq"].append(
                                ln_stats(gate_ap(CHAIN * c + j + LA))
                            )
                        blk = CHAIN * c + j - 1
                        mv_c, rstd_c = S["lnq"].pop(0)
                        psum = None
                        if j >= 1:
                            if (j - 1) % MACRO == 0:
                                S["o4"] = opool.tile(
                                    [W, MACRO * DOUT], res_dt, tag="o4",
                                    name=f"o4_{c}_{j}",
                                )
                                S["c16"] = cpool.tile(
                                    [W, MACRO * DOUT], BF16, tag="c16",
                                    name=f"c16_{c}_{j}",
                                )
                            # prev-window matmuls only need z_prev: the PE
                            # works while this block's znorm is in flight
                            psum = ppool.tile([W, DOUT], F32, tag="psum")
                            for h in range(HEADS):
                                nc.tensor.matmul(
                                    psum[:, h * DHEAD : (h + 1) * DHEAD],
                                    wt_t[:, (2 * h) * W : (2 * h + 1) * W],
                                    S["z_prev"][:, h * DHEAD : (h + 1) * DHEAD],
                                    start=True,
                                    stop=False,
                                )
                        z = ln_norm(gate_ap(CHAIN * c + j), mv_c, rstd_c)
                        if j >= 1:
                            for h in range(HEADS):
                                nc.tensor.matmul(
                                    psum[:, h * DHEAD : (h + 1) * DHEAD],
                                    wt_t[:, (2 * h + 1) * W : (2 * h + 2) * W],
                                    z[:, h * DHEAD : (h + 1) * DHEAD],
                                    start=False,
                                    stop=True,
                                )
                            s = (j - 1) % MACRO
                            nc.scalar.activation(
                                out=S["c16"][:, s * DOUT : (s + 1) * DOUT],
                                in_=psum, func=ident,
                                bias=float(bias_val), scale=1.0,
                            )
                            if s == MACRO - 1:
                                pend.append((
                                    blk // MACRO, S["c16"], S["o4"], 0, MACRO,
                                ))
                        S["z_prev"] = z
                while pend:
                    do_piece(*pend.pop(0))
    if not nc.is_finalized():
        nc.finalize()
    return nc


def _host_prep(weight, bias, ln_beta):
    j = np.arange(2 * W)[None, :]
    i_ = np.arange(W)[:, None]
    mask = (j <= i_ + W).astype(np.float32)          # [W, 2W]
    wm = weight * mask[None]                         # [H, W, 2W]
    wT = np.zeros((W, 2 * HEADS, W), dtype=np.float32)
    for h in range(HEADS):
        wT[:, 2 * h] = wm[h, :, :W].T                # A_h: prev-window cols
        wT[:, 2 * h + 1] = wm[h, :, W:].T            # B_h: current-window cols
    wT = wT.reshape(W, 2 * HEADS * W)

    s_full = wm.sum(-1)                              # [H, W]
    s_first = wm[:, :, W:].sum(-1)

    def consts_for(first_has_prev: bool):
        c = np.zeros((4, _CONSTS_COLS), dtype=np.float32)
        sf = s_full if first_has_prev else s_first
        for u in range(2):
            # lhsT rows: bias[2u], S[2u], bias[2u+1], S[2u+1]
            c[0, _EXR0 + u * W : _EXR0 + (u + 1) * W] = bias[2 * u]
            c[1, _EXR0 + u * W : _EXR0 + (u + 1) * W] = s_full[2 * u]
            c[2, _EXR0 + u * W : _EXR0 + (u + 1) * W] = bias[2 * u + 1]
            c[3, _EXR0 + u * W : _EXR0 + (u + 1) * W] = s_full[2 * u + 1]
            c[0, _EXF0 + u * W : _EXF0 + (u + 1) * W] = bias[2 * u]
            c[1, _EXF0 + u * W : _EXF0 + (u + 1) * W] = sf[2 * u]
            c[2, _EXF0 + u * W : _EXF0 + (u + 1) * W] = bias[2 * u + 1]
            c[3, _EXF0 + u * W : _EXF0 + (u + 1) * W] = sf[2 * u + 1]
            # rhs rows: ind[2u], beta*ind[2u], ind[2u+1], beta*ind[2u+1]
            base = _RHSX0 + u * 512
            beta_u = ln_beta[u * 512 : (u + 1) * 512]
            c[0, base : base + 256] = 1.0
            c[1, base : base + 256] = beta_u[:256]
            c[2, base + 256 : base + 512] = 1.0
            c[3, base + 256 : base + 512] = beta_u[256:]
        return c

    consts_bf = np.ascontiguousarray(wT.astype(ml_dtypes.bfloat16))
    return consts_for(False), consts_for(True), consts_bf


def kernel(x, weight, bias, ln_gamma, ln_beta):
    x = np.ascontiguousarray(x, dtype=np.float32)
    weight = np.asarray(weight, dtype=np.float32)
    bias = np.asarray(bias, dtype=np.float32)
    ln_gamma = np.asarray(ln_gamma, dtype=np.float32)
    ln_beta = np.asarray(ln_beta, dtype=np.float32)

    consts_even, consts_odd, consts_bf = _host_prep(weight, bias, ln_beta)

    bias_uniform = bool(np.all(bias == bias.flat[0]))
    general = not (
        np.all(ln_gamma == 1.0) and np.all(ln_beta == 0.0) and bias_uniform
    )
    bias_val = float(bias.flat[0]) if bias_uniform else 0.0
    key = (general, bias_val)
    if key not in _NC_CACHE:
        _NC_CACHE[key] = _build_nc(general, bias_val)
    nc = _NC_CACHE[key]

    half = N // 2
    nblk = BLK_PER_CORE
    res_np_dt = np.float32 if general else ml_dtypes.bfloat16
    gate_f8 = np.ascontiguousarray(x[:, :, DOUT:]).astype(ml_dtypes.float8_e4m3)

    def to_pmajor(a, nb):
        # [nb*W, DOUT] -> [W, nb*DOUT] (partition-major for flat 2D DMAs)
        return np.ascontiguousarray(
            a.reshape(nb, W, DOUT).transpose(1, 0, 2).reshape(W, nb * DOUT)
        )

    in_maps = []
    for k in range(NCORES):
        bk, hk = k // 2, k % 2
        res_sh = to_pmajor(
            x[bk, hk * half : (hk + 1) * half, :DOUT].astype(res_np_dt), nblk
        )
        if hk == 0:
            halo = np.zeros((W, DOUT), dtype=ml_dtypes.float8_e4m3)
        else:
            halo = gate_f8[bk, half - W : half]
        gate_sh = to_pmajor(
            np.concatenate(
                [halo, gate_f8[bk, hk * half : (hk + 1) * half]], axis=0
            ),
            nblk + 1,
        )
        m = {
            "res_sh": res_sh,
            "gate_sh": gate_sh,
            "consts4": consts_odd if hk == 1 else consts_even,
            "consts_bf": consts_bf,
        }
        if general:
            m["gamma"] = ln_gamma
        in_maps.append(m)

    global _last_in_maps
    _last_in_maps = in_maps

    res = run_bass_kernel_spmd(nc, in_maps, list(range(NCORES)))

    out = np.empty((B, N, DOUT), dtype=np.float32)
    for k in range(NCORES):
        bk, hk = k // 2, k % 2
        o = res.results[k]["out"]  # [W, nblk*DOUT] partition-major
        o = o.reshape(W, nblk, DOUT).transpose(1, 0, 2).reshape(half, DOUT)
        out[bk, hk * half : (hk + 1) * half] = o.astype(np.float32)
    return out
